# revision 1
# baseline (speedup 1.0000x reference)
"""Trainium2 Bass kernel for the StyleGAN2-style upsampling conv layer.

Reference computation (per batch image):
  y = conv_transpose2d(x, w * s, stride=2)          # [512, 129, 129]
  y = depthwise_fir(y, outer([1,3,3,1])/8 * 4)      # [512, 128, 128]
  y = y + noise * strength
  y = clamp(lrelu(y + bias) * sqrt(2), +-256)

Implementation: the transposed conv + FIR fuse into a single 6x6 kernel
G6 = f2 (*) (w*s) on the stride-2 upsampled grid; polyphase decomposition
turns that into four ordinary 3x3 convolutions over the 64x64 input (one
per output-pixel parity (alpha, beta)).  Each conv maps to PE matmuls
contracting input channels (4 tiles of 128), with the 9 taps x 4 ci-tiles
accumulated in PSUM.  The noise term is a 37th accumulated matmul with
K=1 (broadcast over out-channel partitions).  Epilogue: ScalarE Prelu
(scale sqrt2, per-channel bias, alpha 0.2) + VectorE fused clamp.

Sharding: pure data parallel - one batch image per NeuronCore (N=8).
"""

import numpy as np
import ml_dtypes

N, CIN, COUT, RES, KK, UP = 8, 512, 512, 128, 3, 2
IN_RES = RES // UP  # 64
P = 128
NCT = CIN // P   # 4 ci tiles
NOT = COUT // P  # 4 co tiles
SQRT2 = float(np.sqrt(2.0))
CLAMP = 256.0
LRELU_SLOPE = 0.2

_CACHE = {}


def _build_program():
    import concourse.mybir as mybir
    import concourse.tile as tile
    from concourse import bacc

    nc = bacc.Bacc(None, target_bir_lowering=False)

    xp = nc.declare_dram_parameter("xp", [NCT, P, 66, 66], mybir.dt.bfloat16, isOutput=False)
    wt = nc.declare_dram_parameter("wt", [NOT, 36, NCT, P, P], mybir.dt.bfloat16, isOutput=False)
    nz = nc.declare_dram_parameter("nz", [1, 2, 2, 64, 64], mybir.dt.bfloat16, isOutput=False)
    sn = nc.declare_dram_parameter("sn", [1, P], mybir.dt.bfloat16, isOutput=False)
    bv = nc.declare_dram_parameter("bv", [P, NOT], mybir.dt.float32, isOutput=False)
    out = nc.declare_dram_parameter("out", [COUT, RES, RES], mybir.dt.float32, isOutput=True)

    out_r = out[:].rearrange("c (r t) w -> c r t w", t=2)  # row = 2r + t

    with tile.TileContext(nc) as tc:
        with (
            tc.tile_pool(name="const", bufs=1) as const,
            tc.tile_pool(name="wpool", bufs=2) as wpool,
            tc.tile_pool(name="pspool", bufs=6, space="PSUM") as pspool,
            tc.tile_pool(name="stpool", bufs=3) as stpool,
        ):
            x_sb = const.tile([P, NCT, 66, 66], mybir.dt.bfloat16)
            nz_sb = const.tile([1, 2, 2, 64, 64], mybir.dt.bfloat16)
            sn_sb = const.tile([1, P], mybir.dt.bfloat16)
            bv_sb = const.tile([P, NOT], mybir.dt.float32)
            b2_sb = const.tile([P, NOT], mybir.dt.float32)

            for ct in range(NCT):
                nc.sync.dma_start(out=x_sb[:, ct], in_=xp[ct])
            nc.sync.dma_start(out=nz_sb[:], in_=nz[:])
            nc.sync.dma_start(out=sn_sb[:], in_=sn[:])
            nc.sync.dma_start(out=bv_sb[:], in_=bv[:])
            nc.vector.tensor_scalar_mul(b2_sb[:], bv_sb[:], SQRT2)

            for co_t in range(NOT):
                w_sb = wpool.tile([P, 36, NCT, P], mybir.dt.bfloat16)
                for ct in range(NCT):
                    nc.sync.dma_start(
                        out=w_sb[:, :, ct, :],
                        in_=wt[co_t, :, ct].rearrange("t k m -> k t m"),
                    )
                for alpha in range(2):
                    for blk in range(8):
                        a0 = 8 * blk
                        zst = stpool.tile([P, 8, 64, 2], mybir.dt.float32)
                        for beta in range(2):
                            psum = pspool.tile([P, 8, 64], mybir.dt.float32)
                            first = True
                            for e1 in (-1, 0, 1):
                                for e2 in (-1, 0, 1):
                                    tap = alpha * 18 + beta * 9 + (e1 + 1) * 3 + (e2 + 1)
                                    for ct in range(NCT):
                                        nc.tensor.matmul(
                                            psum[:],
                                            w_sb[:, tap, ct, :],
                                            x_sb[:, ct, 1 + a0 + e1 : 9 + a0 + e1, 1 + e2 : 65 + e2],
                                            start=first,
                                            stop=False,
                                        )
                                        first = False
                            nc.tensor.matmul(
                                psum[:],
                                sn_sb[0:1, :],
                                nz_sb[0:1, alpha, beta, a0 : a0 + 8, :],
                                start=False,
                                stop=True,
                            )
                            nc.scalar.activation(
                                zst[:, :, :, beta],
                                psum[:],
                                mybir.ActivationFunctionType.Prelu,
                                bias=b2_sb[:, co_t : co_t + 1],
                                scale=SQRT2,
                                alpha=LRELU_SLOPE,
                            )
                        nc.vector.tensor_scalar(
                            zst[:],
                            zst[:],
                            CLAMP,
                            -CLAMP,
                            op0=mybir.AluOpType.min,
                            op1=mybir.AluOpType.max,
                        )
                        nc.sync.dma_start(
                            out=out_r[co_t * P : (co_t + 1) * P, a0 : a0 + 8, alpha, :],
                            in_=zst[:].rearrange("p r c t -> p r (c t)"),
                        )

    nc.finalize()
    return nc


def _prep_weights(weight: np.ndarray) -> np.ndarray:
    """Effective 6x6 kernel (conv-transpose x FIR fused), polyphase-split to
    36 [ci, co] matmul weight matrices laid out [co_t, tap, ci_t, ci, co]."""
    w = weight.astype(np.float64) / np.sqrt(CIN * KK * KK)
    f1 = np.array([1.0, 3.0, 3.0, 1.0]) / 8.0
    f2 = np.outer(f1, f1) * (UP * UP)
    G6 = np.zeros((COUT, CIN, 6, 6))
    for m1 in range(-2, 4):
        for m2 in range(-2, 4):
            acc = np.zeros((COUT, CIN))
            for u1 in range(4):
                r1 = m1 + u1 - 1
                if not (0 <= r1 < 3):
                    continue
                for u2 in range(4):
                    r2 = m2 + u2 - 1
                    if not (0 <= r2 < 3):
                        continue
                    acc += f2[u1, u2] * w[:, :, r1, r2]
            G6[:, :, m1 + 2, m2 + 2] = acc

    WT = np.zeros((NOT, 36, NCT, P, P), np.float32)
    for alpha in range(2):
        for beta in range(2):
            for e1 in (-1, 0, 1):
                for e2 in (-1, 0, 1):
                    tap = alpha * 18 + beta * 9 + (e1 + 1) * 3 + (e2 + 1)
                    M = G6[:, :, alpha + 2 - 2 * e1, beta + 2 - 2 * e2]  # [CO, CI]
                    MT = np.ascontiguousarray(M.T, np.float32)  # [CI, CO] = lhsT
                    WT[:, tap] = (
                        MT.reshape(NCT, P, NOT, P).transpose(2, 0, 1, 3)
                    )
    return WT.astype(ml_dtypes.bfloat16)


def kernel(x, weight, bias, noise_const, noise_strength):
    from concourse.bass_utils import run_bass_kernel_spmd

    x = np.asarray(x)
    weight = np.asarray(weight)
    bias = np.asarray(bias, np.float32)
    noise_const = np.asarray(noise_const)
    noise_strength = np.asarray(noise_strength)

    if "nc" not in _CACHE:
        _CACHE["nc"] = _build_program()
    nc = _CACHE["nc"]

    WT = _prep_weights(weight)

    nzp = np.empty((1, 2, 2, 64, 64), np.float32)
    for alpha in range(2):
        for beta in range(2):
            nzp[0, alpha, beta] = noise_const[alpha::2, beta::2]
    nzp = nzp.astype(ml_dtypes.bfloat16)
    snv = np.full((1, P), float(noise_strength), np.float32).astype(ml_dtypes.bfloat16)
    bvv = np.ascontiguousarray(bias.reshape(NOT, P).T, np.float32)  # [P, NOT]

    in_maps = []
    for n in range(N):
        xpad = np.zeros((NCT, P, 66, 66), np.float32)
        xpad[:, :, 1:65, 1:65] = x[n].reshape(NCT, P, 64, 64)
        in_maps.append(
            {
                "xp": xpad.astype(ml_dtypes.bfloat16),
                "wt": WT,
                "nz": nzp,
                "sn": snv,
                "bv": bvv,
            }
        )

    res = run_bass_kernel_spmd(nc, in_maps, core_ids=list(range(N)))
    outp = np.stack([res.results[n]["out"] for n in range(N)], axis=0)
    return outp.astype(np.float32)


# revision 5
# speedup vs baseline: 1.7876x; 1.7876x over previous
"""Trainium2 Bass kernel for the StyleGAN2-style upsampling conv layer.

Reference computation (per batch image):
  y = conv_transpose2d(x, w * s, stride=2)          # [512, 129, 129]
  y = depthwise_fir(y, outer([1,3,3,1])/8 * 4)      # [512, 128, 128]
  y = y + noise * strength
  y = clamp(lrelu(y + bias) * sqrt(2), +-256)

Implementation (per core = one batch image, pure data parallel):
  * The horizontal FIR axis is fused into the conv weights: GH' =
    (w*s) (*)_h f1, polyphase-split over output-pixel parity.  The
    transposed conv then becomes, for each upsampled row i and column
    parity beta, a matmul accumulation over (vertical tap rv, horizontal
    tap e2, ci-tile) - 18 distinct [ci,co] weight matrices, 12 matmuls
    per odd row group / 24 per even row group into PSUM.
  * q rows (the H-filtered upsampled-grid conv output) are copied
    PSUM->SBUF as bf16 by ScalarE.
  * The vertical 4-tap FIR [1,3,3,1] (x 1/4 folded into GH') becomes 4
    shifted-row adds on VectorE: z = (A + D) + 3*(B + C) + noise.
  * Epilogue: ScalarE Prelu(scale sqrt2, per-channel bias*sqrt2,
    alpha 0.2) writing column-interleaved fp32, VectorE fused clamp,
    DMA out with row interleave.
"""

import numpy as np
import ml_dtypes

N, CIN, COUT, RES, KK, UP = 8, 512, 512, 128, 3, 2
IN_RES = RES // UP  # 64
P = 128
NCT = CIN // P   # 4 ci tiles
NOT = COUT // P  # 4 co tiles
SQRT2 = float(np.sqrt(2.0))
CLAMP = 256.0
LRELU_SLOPE = 0.2

_CACHE = {}

# vertical taps per row parity: (rv, e1) with x row = a + e1
VTAPS = {0: ((0, 0), (2, -1)), 1: ((1, 0),)}


def _build_program():
    import concourse.mybir as mybir
    import concourse.tile as tile
    from concourse import bacc

    bf16 = mybir.dt.bfloat16
    f32 = mybir.dt.float32

    nc = bacc.Bacc(None, target_bir_lowering=False)

    xp = nc.declare_dram_parameter("xp", [NCT, P, 66, 66], bf16, isOutput=False)
    # tap index: rv*6 + beta*3 + (e2+1)
    wt = nc.declare_dram_parameter("wt", [NOT, 18, NCT, P, P], bf16, isOutput=False)
    # noise, parity-split rows, concat cols: [parity, a, (beta,32->64c)]
    nzr = nc.declare_dram_parameter("nzr", [1, 2, 64, 128], bf16, isOutput=False)
    sn = nc.declare_dram_parameter("sn", [1, 1], f32, isOutput=False)
    bv = nc.declare_dram_parameter("bv", [P, NOT], f32, isOutput=False)
    out = nc.declare_dram_parameter("out", [COUT, RES, RES], f32, isOutput=True)

    out_r = out[:].rearrange("c (r t) w -> c r t w", t=2)  # out row = 2r + t

    with tile.TileContext(nc) as tc:
        with (
            tc.tile_pool(name="const", bufs=1) as const,
            tc.tile_pool(name="wpool", bufs=2) as wpool,
            tc.tile_pool(name="qpool", bufs=1) as qpool,
            tc.tile_pool(name="pspool", bufs=6, space="PSUM") as pspool,
            tc.tile_pool(name="scratch", bufs=2) as scratch,
            tc.tile_pool(name="stpool", bufs=3) as stpool,
        ):
            x_sb = const.tile([P, NCT, 66, 66], bf16)
            nb_sb = const.tile([P, 2, 64, 128], bf16)  # broadcast noise * strength
            sn_sb = const.tile([P, 1], f32)
            bv_sb = const.tile([P, NOT], f32)
            b2_sb = const.tile([P, NOT], f32)

            for ct in range(NCT):
                nc.sync.dma_start(out=x_sb[:, ct], in_=xp[ct])
            nc.sync.dma_start(out=nb_sb[:], in_=nzr[:].partition_broadcast(P))
            nc.sync.dma_start(out=sn_sb[:], in_=sn[:].partition_broadcast(P))
            nc.sync.dma_start(out=bv_sb[:], in_=bv[:])
            nc.vector.tensor_scalar_mul(b2_sb[:], bv_sb[:], SQRT2)
            # noise * strength (per-partition scalar AP)
            nc.vector.tensor_scalar_mul(nb_sb[:], nb_sb[:], sn_sb[:])

            for co_t in range(NOT):
                w_sb = wpool.tile([P, 18, NCT, P], bf16)
                for ct in range(NCT):
                    nc.sync.dma_start(
                        out=w_sb[:, :, ct, :],
                        in_=wt[co_t, :, ct].rearrange("t k m -> k t m"),
                    )

                # q planes (bf16): q_e[a] = q row 2a (a in 0..64);
                # q_o[i] = q row 2(i-1)+1 (odd rows for a = -1..64)
                q_e = qpool.tile([P, 65, 128], bf16)
                q_o = qpool.tile([P, 66, 128], bf16)

                for parity in range(2):
                    nrows_tot = 65 if parity == 0 else 66
                    a_base = 0 if parity == 0 else -1
                    qdst = q_e if parity == 0 else q_o
                    taps_v = VTAPS[parity]
                    for beta in range(2):
                        for g in range((nrows_tot + 7) // 8):
                            i0 = 8 * g
                            rows = min(8, nrows_tot - i0)
                            a0 = a_base + i0
                            psq = pspool.tile([P, 8, 64], f32, tag="ps")
                            n_mm = len(taps_v) * 3 * NCT
                            k = 0
                            for rv, e1 in taps_v:
                                for e2 in (-1, 0, 1):
                                    tap = rv * 6 + beta * 3 + (e2 + 1)
                                    for ct in range(NCT):
                                        nc.tensor.matmul(
                                            psq[:, :rows, :],
                                            w_sb[:, tap, ct, :],
                                            x_sb[
                                                :,
                                                ct,
                                                1 + a0 + e1 : 1 + a0 + e1 + rows,
                                                1 + e2 : 65 + e2,
                                            ],
                                            start=(k == 0),
                                            stop=(k == n_mm - 1),
                                        )
                                        k += 1
                            nc.scalar.copy(
                                qdst[:, i0 : i0 + rows, beta * 64 : beta * 64 + 64],
                                psq[:, :rows, :],
                            )

                # combine 16-row blocks: z_even / z_odd from shifted q rows
                for t in range(4):
                    a0 = 16 * t
                    for parity in range(2):
                        if parity == 0:
                            A = q_o[:, a0 : a0 + 16, :]
                            B = q_e[:, a0 : a0 + 16, :]
                            C = q_o[:, a0 + 1 : a0 + 17, :]
                            D = q_e[:, a0 + 1 : a0 + 17, :]
                        else:
                            A = q_e[:, a0 : a0 + 16, :]
                            B = q_o[:, a0 + 1 : a0 + 17, :]
                            C = q_e[:, a0 + 1 : a0 + 17, :]
                            D = q_o[:, a0 + 2 : a0 + 18, :]
                        E = scratch.tile([P, 16, 128], bf16, tag="E")
                        F = scratch.tile([P, 16, 128], bf16, tag="F")
                        T = scratch.tile([P, 16, 128], bf16, tag="T")
                        U = scratch.tile([P, 16, 128], bf16, tag="U")
                        nc.vector.tensor_add(E[:], A, D)
                        nc.vector.tensor_add(F[:], B, C)
                        # T = 3*F + E
                        nc.vector.scalar_tensor_tensor(
                            T[:], F[:], 3.0, E[:],
                            op0=mybir.AluOpType.mult, op1=mybir.AluOpType.add,
                        )
                        nc.vector.tensor_add(
                            U[:], T[:], nb_sb[:, parity, a0 : a0 + 16, :]
                        )
                        zf = stpool.tile([P, 16, 128], f32, tag="zf")
                        nc.scalar.activation(
                            zf[:].rearrange("p r (c t) -> p r t c", t=2),
                            U[:],
                            mybir.ActivationFunctionType.Prelu,
                            bias=b2_sb[:, co_t : co_t + 1],
                            scale=SQRT2,
                            alpha=LRELU_SLOPE,
                        )
                        nc.vector.tensor_scalar(
                            zf[:],
                            zf[:],
                            CLAMP,
                            -CLAMP,
                            op0=mybir.AluOpType.min,
                            op1=mybir.AluOpType.max,
                        )
                        nc.sync.dma_start(
                            out=out_r[
                                co_t * P : (co_t + 1) * P, a0 : a0 + 16, parity, :
                            ],
                            in_=zf[:],
                        )

    nc.finalize()
    return nc


def _prep_weights(weight: np.ndarray) -> np.ndarray:
    """GH'[o,c,rv,m2+2] = (1/4) * sum_u2 2*f1[u2] * w_s[o,c,rv,m2+u2-1],
    laid out as 18 lhsT [ci,co] matrices: tap = rv*6 + beta*3 + (e2+1)
    maps to GH'[:, :, rv, beta+2-2*e2]."""
    w = weight.astype(np.float64) / np.sqrt(CIN * KK * KK)
    f1 = np.array([1.0, 3.0, 3.0, 1.0]) / 8.0
    GH = np.zeros((COUT, CIN, 3, 6))
    for m2 in range(-2, 4):
        acc = np.zeros((COUT, CIN, 3))
        for u2 in range(4):
            r2 = m2 + u2 - 1
            if not (0 <= r2 < 3):
                continue
            acc += (2.0 * f1[u2]) * w[:, :, :, r2]
        GH[:, :, :, m2 + 2] = acc
    # DVE combine uses raw [1,3,3,1]; true vertical filter is 2*f1 =
    # [1,3,3,1]/4, so fold 1/4 here.
    GH *= 0.25

    WT = np.zeros((NOT, 18, NCT, P, P), np.float32)
    for rv in range(3):
        for beta in range(2):
            for e2 in (-1, 0, 1):
                tap = rv * 6 + beta * 3 + (e2 + 1)
                M = GH[:, :, rv, beta + 2 - 2 * e2]  # [CO, CI]
                MT = np.ascontiguousarray(M.T, np.float32)  # lhsT [CI, CO]
                WT[:, tap] = MT.reshape(NCT, P, NOT, P).transpose(2, 0, 1, 3)
    return WT.astype(ml_dtypes.bfloat16)


def _prep_inputs(x, weight, bias, noise_const, noise_strength):
    WT = _prep_weights(weight)
    noise = np.asarray(noise_const, np.float32)
    nzp = np.empty((1, 2, 64, 128), np.float32)
    for parity in range(2):
        nzp[0, parity, :, 0:64] = noise[parity::2, 0::2]
        nzp[0, parity, :, 64:128] = noise[parity::2, 1::2]
    nzp = nzp.astype(ml_dtypes.bfloat16)
    snv = np.asarray(noise_strength, np.float32).reshape(1, 1)
    bvv = np.ascontiguousarray(
        np.asarray(bias, np.float32).reshape(NOT, P).T
    )  # [P, NOT]

    in_maps = []
    for n in range(N):
        xpad = np.zeros((NCT, P, 66, 66), np.float32)
        xpad[:, :, 1:65, 1:65] = np.asarray(x[n], np.float32).reshape(NCT, P, 64, 64)
        in_maps.append(
            {
                "xp": xpad.astype(ml_dtypes.bfloat16),
                "wt": WT,
                "nzr": nzp,
                "sn": snv,
                "bv": bvv,
            }
        )
    return in_maps


def kernel(x, weight, bias, noise_const, noise_strength):
    from concourse.bass_utils import run_bass_kernel_spmd

    if "nc" not in _CACHE:
        _CACHE["nc"] = _build_program()
    nc = _CACHE["nc"]

    in_maps = _prep_inputs(x, weight, bias, noise_const, noise_strength)
    res = run_bass_kernel_spmd(nc, in_maps, core_ids=list(range(N)))
    outp = np.stack([res.results[n]["out"] for n in range(N)], axis=0)
    return outp.astype(np.float32)


# revision 6
# speedup vs baseline: 1.9372x; 1.0837x over previous
"""Trainium2 Bass kernel for the StyleGAN2-style upsampling conv layer.

Reference computation (per batch image):
  y = conv_transpose2d(x, w * s, stride=2)          # [512, 129, 129]
  y = depthwise_fir(y, outer([1,3,3,1])/8 * 4)      # [512, 128, 128]
  y = y + noise * strength
  y = clamp(lrelu(y + bias) * sqrt(2), +-256)

Implementation (per core = one batch image, pure data parallel):
  * The horizontal FIR axis is fused into the conv weights: GH' =
    (w*s) (*)_h f1, polyphase-split over output-pixel parity.  The
    transposed conv then becomes, for each upsampled row i and column
    parity beta, a matmul accumulation over (vertical tap rv, horizontal
    tap e2, ci-tile) - 18 distinct [ci,co] weight matrices, 12 matmuls
    per odd row group / 24 per even row group into PSUM.
  * q rows (the H-filtered upsampled-grid conv output) are copied
    PSUM->SBUF as bf16 by ScalarE.
  * The vertical 4-tap FIR [1,3,3,1] (x 1/4 folded into GH') becomes 4
    shifted-row adds on VectorE: z = (A + D) + 3*(B + C) + noise.
  * Epilogue: ScalarE Prelu(scale sqrt2, per-channel bias*sqrt2,
    alpha 0.2) writing column-interleaved fp32, VectorE fused clamp,
    DMA out with row interleave.
"""

import numpy as np
import ml_dtypes

N, CIN, COUT, RES, KK, UP = 8, 512, 512, 128, 3, 2
IN_RES = RES // UP  # 64
P = 128
NCT = CIN // P   # 4 ci tiles
NOT = COUT // P  # 4 co tiles
SQRT2 = float(np.sqrt(2.0))
CLAMP = 256.0
LRELU_SLOPE = 0.2

_CACHE = {}

# vertical taps per row parity: (rv, e1) with x row = a + e1
VTAPS = {0: ((0, 0), (2, -1)), 1: ((1, 0),)}


def _build_program():
    import concourse.mybir as mybir
    import concourse.tile as tile
    from concourse import bacc

    bf16 = mybir.dt.bfloat16
    f32 = mybir.dt.float32

    nc = bacc.Bacc(None, target_bir_lowering=False)

    xp = nc.declare_dram_parameter("xp", [NCT, P, 66, 66], bf16, isOutput=False)
    # tap index: rv*6 + beta*3 + (e2+1)
    wt = nc.declare_dram_parameter("wt", [NOT, 18, NCT, P, P], bf16, isOutput=False)
    # noise, parity-split rows, concat cols: [parity, a, (beta,32->64c)]
    nzr = nc.declare_dram_parameter("nzr", [1, 2, 64, 128], bf16, isOutput=False)
    sn = nc.declare_dram_parameter("sn", [1, 1], f32, isOutput=False)
    bv = nc.declare_dram_parameter("bv", [P, NOT], f32, isOutput=False)
    out = nc.declare_dram_parameter("out", [COUT, RES, RES], f32, isOutput=True)

    out_r = out[:].rearrange("c (r t) w -> c r t w", t=2)  # out row = 2r + t

    with tile.TileContext(nc) as tc:
        with (
            tc.tile_pool(name="const", bufs=1) as const,
            tc.tile_pool(name="wpool", bufs=2) as wpool,
            tc.tile_pool(name="qpool", bufs=1) as qpool,
            tc.tile_pool(name="pspool", bufs=6, space="PSUM") as pspool,
            tc.tile_pool(name="scratch", bufs=2) as scratch,
            tc.tile_pool(name="stpool", bufs=3) as stpool,
        ):
            x_sb = const.tile([P, NCT, 66, 66], bf16)
            nb_sb = const.tile([P, 2, 64, 128], bf16)  # broadcast noise * strength
            sn_sb = const.tile([P, 1], f32)
            bv_sb = const.tile([P, NOT], f32)
            b2_sb = const.tile([P, NOT], f32)

            for ct in range(NCT):
                nc.sync.dma_start(out=x_sb[:, ct], in_=xp[ct])
            nc.sync.dma_start(out=nb_sb[:], in_=nzr[:].partition_broadcast(P))
            nc.sync.dma_start(out=sn_sb[:], in_=sn[:].partition_broadcast(P))
            nc.sync.dma_start(out=bv_sb[:], in_=bv[:])
            nc.vector.tensor_scalar_mul(b2_sb[:], bv_sb[:], SQRT2)
            # noise * strength (per-partition scalar AP)
            nc.vector.tensor_scalar_mul(nb_sb[:], nb_sb[:], sn_sb[:])

            for co_t in range(NOT):
                w_sb = wpool.tile([P, 18, NCT, P], bf16)
                for ct in range(NCT):
                    nc.sync.dma_start(
                        out=w_sb[:, :, ct, :],
                        in_=wt[co_t, :, ct].rearrange("t k m -> k t m"),
                    )

                # q planes (bf16): q_e[a] = q row 2a (a in 0..64);
                # q_o[i] = q row 2(i-1)+1 (odd rows for a = -1..64)
                q_e = qpool.tile([P, 65, 128], bf16)
                q_o = qpool.tile([P, 66, 128], bf16)

                def produce_group(parity, beta, g):
                    nrows_tot = 65 if parity == 0 else 66
                    a_base = 0 if parity == 0 else -1
                    qdst = q_e if parity == 0 else q_o
                    taps_v = VTAPS[parity]
                    i0 = 8 * g
                    rows = min(8, nrows_tot - i0)
                    if rows <= 0:
                        return
                    a0 = a_base + i0
                    psq = pspool.tile([P, 8, 64], f32, tag="ps", name="psq")
                    n_mm = len(taps_v) * 3 * NCT
                    k = 0
                    for rv, e1 in taps_v:
                        for e2 in (-1, 0, 1):
                            tap = rv * 6 + beta * 3 + (e2 + 1)
                            for ct in range(NCT):
                                nc.tensor.matmul(
                                    psq[:, :rows, :],
                                    w_sb[:, tap, ct, :],
                                    x_sb[
                                        :,
                                        ct,
                                        1 + a0 + e1 : 1 + a0 + e1 + rows,
                                        1 + e2 : 65 + e2,
                                    ],
                                    start=(k == 0),
                                    stop=(k == n_mm - 1),
                                )
                                k += 1
                    nc.scalar.copy(
                        qdst[:, i0 : i0 + rows, beta * 64 : beta * 64 + 64],
                        psq[:, :rows, :],
                    )

                # interleave group production across parity/beta so the row
                # combines can start while later groups are still on the PE
                for g in range(9):
                    for parity in range(2):
                        for beta in range(2):
                            produce_group(parity, beta, g)

                # combine 16-row blocks: z_even / z_odd from shifted q rows
                for t in range(4):
                    a0 = 16 * t
                    for parity in range(2):
                        if parity == 0:
                            A = q_o[:, a0 : a0 + 16, :]
                            B = q_e[:, a0 : a0 + 16, :]
                            C = q_o[:, a0 + 1 : a0 + 17, :]
                            D = q_e[:, a0 + 1 : a0 + 17, :]
                        else:
                            A = q_e[:, a0 : a0 + 16, :]
                            B = q_o[:, a0 + 1 : a0 + 17, :]
                            C = q_e[:, a0 + 1 : a0 + 17, :]
                            D = q_o[:, a0 + 2 : a0 + 18, :]
                        E = scratch.tile([P, 16, 128], bf16, tag="E")
                        F = scratch.tile([P, 16, 128], bf16, tag="F")
                        T = scratch.tile([P, 16, 128], bf16, tag="T")
                        U = scratch.tile([P, 16, 128], bf16, tag="U")
                        nc.vector.tensor_add(E[:], A, D)
                        nc.vector.tensor_add(F[:], B, C)
                        # T = 3*F + E
                        nc.vector.scalar_tensor_tensor(
                            T[:], F[:], 3.0, E[:],
                            op0=mybir.AluOpType.mult, op1=mybir.AluOpType.add,
                        )
                        nc.vector.tensor_add(
                            U[:], T[:], nb_sb[:, parity, a0 : a0 + 16, :]
                        )
                        zf = stpool.tile([P, 16, 128], f32, tag="zf")
                        nc.scalar.activation(
                            zf[:].rearrange("p r (c t) -> p r t c", t=2),
                            U[:],
                            mybir.ActivationFunctionType.Prelu,
                            bias=b2_sb[:, co_t : co_t + 1],
                            scale=SQRT2,
                            alpha=LRELU_SLOPE,
                        )
                        nc.vector.tensor_scalar(
                            zf[:],
                            zf[:],
                            CLAMP,
                            -CLAMP,
                            op0=mybir.AluOpType.min,
                            op1=mybir.AluOpType.max,
                        )
                        nc.sync.dma_start(
                            out=out_r[
                                co_t * P : (co_t + 1) * P, a0 : a0 + 16, parity, :
                            ],
                            in_=zf[:],
                        )

    nc.finalize()
    return nc


def _prep_weights(weight: np.ndarray) -> np.ndarray:
    """GH'[o,c,rv,m2+2] = (1/4) * sum_u2 2*f1[u2] * w_s[o,c,rv,m2+u2-1],
    laid out as 18 lhsT [ci,co] matrices: tap = rv*6 + beta*3 + (e2+1)
    maps to GH'[:, :, rv, beta+2-2*e2]."""
    w = weight.astype(np.float64) / np.sqrt(CIN * KK * KK)
    f1 = np.array([1.0, 3.0, 3.0, 1.0]) / 8.0
    GH = np.zeros((COUT, CIN, 3, 6))
    for m2 in range(-2, 4):
        acc = np.zeros((COUT, CIN, 3))
        for u2 in range(4):
            r2 = m2 + u2 - 1
            if not (0 <= r2 < 3):
                continue
            acc += (2.0 * f1[u2]) * w[:, :, :, r2]
        GH[:, :, :, m2 + 2] = acc
    # DVE combine uses raw [1,3,3,1]; true vertical filter is 2*f1 =
    # [1,3,3,1]/4, so fold 1/4 here.
    GH *= 0.25

    WT = np.zeros((NOT, 18, NCT, P, P), np.float32)
    for rv in range(3):
        for beta in range(2):
            for e2 in (-1, 0, 1):
                tap = rv * 6 + beta * 3 + (e2 + 1)
                M = GH[:, :, rv, beta + 2 - 2 * e2]  # [CO, CI]
                MT = np.ascontiguousarray(M.T, np.float32)  # lhsT [CI, CO]
                WT[:, tap] = MT.reshape(NCT, P, NOT, P).transpose(2, 0, 1, 3)
    return WT.astype(ml_dtypes.bfloat16)


def _prep_inputs(x, weight, bias, noise_const, noise_strength):
    WT = _prep_weights(weight)
    noise = np.asarray(noise_const, np.float32)
    nzp = np.empty((1, 2, 64, 128), np.float32)
    for parity in range(2):
        nzp[0, parity, :, 0:64] = noise[parity::2, 0::2]
        nzp[0, parity, :, 64:128] = noise[parity::2, 1::2]
    nzp = nzp.astype(ml_dtypes.bfloat16)
    snv = np.asarray(noise_strength, np.float32).reshape(1, 1)
    bvv = np.ascontiguousarray(
        np.asarray(bias, np.float32).reshape(NOT, P).T
    )  # [P, NOT]

    in_maps = []
    for n in range(N):
        xpad = np.zeros((NCT, P, 66, 66), np.float32)
        xpad[:, :, 1:65, 1:65] = np.asarray(x[n], np.float32).reshape(NCT, P, 64, 64)
        in_maps.append(
            {
                "xp": xpad.astype(ml_dtypes.bfloat16),
                "wt": WT,
                "nzr": nzp,
                "sn": snv,
                "bv": bvv,
            }
        )
    return in_maps


def kernel(x, weight, bias, noise_const, noise_strength):
    from concourse.bass_utils import run_bass_kernel_spmd

    if "nc" not in _CACHE:
        _CACHE["nc"] = _build_program()
    nc = _CACHE["nc"]

    in_maps = _prep_inputs(x, weight, bias, noise_const, noise_strength)
    res = run_bass_kernel_spmd(nc, in_maps, core_ids=list(range(N)))
    outp = np.stack([res.results[n]["out"] for n in range(N)], axis=0)
    return outp.astype(np.float32)
